# revision 72
# baseline (speedup 1.0000x reference)
"""Trainium2 Bass kernel for nn_MultiHeadAttention_76587856823057.

Sharding: (batch, query-half) -> 8 cores, zero collectives.
Per core: b fixed, queries TQ=1024 (half of T), all H=16 heads, all TK=2048 keys.

v4 design notes (delta over v2, which measured 620us; v4 measures ~533-542us):
 - warm-up matmuls are K=128 (full contraction rows): v2's K=1 warm-ups never
   registered as PE activity for the HAM clock gate, so the first ~32us ran
   at 1.2 GHz.  They read uninitialized qhT (no input deps, WAR only).
 - Q-proj runs as two half-contractions (ki 0-3, then 4-7 + DVE add folding
   the bf16 half-A partials back in) so it starts on the FIRST halves of the
   wq/qT DMAs (~14us); attn_sb doubles as the half-A scratch (unused until
   pass-1 kt2).  Startup is HBM-bandwidth-bound (~13MB critical set), so the
   rings are prioritized: wq/qT lead the two HWDGE rings, wk + pair-0 kT
   staging + wv + mask ride the gpsimd SWDGE ring.
 - kT staging is split load/mm with loads fired ~8 kt-iterations ahead on
   the sync ring (v2 issued them just-in-time on the PE-consuming path and
   head-of-line blocked all engines ~2x1.7us per pass).
 - kproj matmul groups run at the pass TAIL + kt3 of the next pass: they
   fill the PE idle while the last ~4 exps of the pass drain, which also
   stops the HAM clock gate re-throttling at every pass boundary.
 - pv0/pv1 merged into one [65, 2, 512] psum tile -> evac: one [1,1024]
   rowsum copy + approx-reciprocal, raw (unnormalized, scale-safe) bf16 PV
   move-out at kt2 frees the psum, one full-width gpsimd broadcast and
   in-place normalize muls at kt6.
 - khp psum evacuation on the scalar engine (DVE is the steady-state
   bottleneck: mask muls + evac ~18.8us/pass vs ACT exp 16.8us/pass).
 - Wo reuses the wk SBUF slot (allocated after pair-7 kproj, pass 13);
   phase C allocates its O-proj psum from the qk pool and is emitted right
   after pass 15 so the qh=0 output projection overlaps the tail.
 - everything else as v2: softmax shift-invariance (no global max), one mask
   multiply after exp, PE row-tiled QK (2 heads concurrent), rowsums via a
   ones-column in the V projection (M=65 PV), bf16 compute, biases via
   ones-row K=1 matmuls, software-pipelined evacuation.
 - measured engine budgets/pass (steady): DVE ~18.8us (91%), PE ~18us (87%),
   ACT exp 16x1.05us (81%); pass cadence ~20.7us.  Known remaining headroom:
   K/V-proj duplication across the q-half core pairs (~60us PE, needs a
   pair collective), exp [128,2048] batching (needs 2 more PSUM banks).

Self-contained: hardcodes all shapes; no sibling imports.
"""

import os
import numpy as np

import concourse.bass as bass
from concourse import bacc
import concourse.mybir as mybir
from concourse.tile import TileContext
from concourse.bass_utils import run_bass_kernel_spmd

F32 = mybir.dt.float32
BF16 = mybir.dt.bfloat16
AF = mybir.ActivationFunctionType

B, T, D, H, DK = 4, 2048, 1024, 16, 64
TQ = T // 2          # queries per core
TK = T               # keys per core
NCORES = 8
NPAIR = H // 2       # 8 head pairs
NFT = D // 128       # 8 feature tiles
NKT = TK // 128      # 16 key tiles
VEXT = H * (DK + 1)  # 1040: per-head [64 v-cols + ones col]

_LAST_RESULTS = {}


def build_program(nc: bass.Bass, trivial_affine: bool = False,
                  trivial_bias: bool = False):
    # ---- per-core DRAM I/O ----
    qT = nc.dram_tensor("qT", [D + 1, TQ], BF16, kind="ExternalInput").ap()
    kT = nc.dram_tensor("kT", [D + 1, TK], BF16, kind="ExternalInput").ap()
    vT = nc.dram_tensor("vT", [D + 1, TK], BF16, kind="ExternalInput").ap()
    wq = nc.dram_tensor("wq", [D + 1, D], BF16, kind="ExternalInput").ap()
    wk = nc.dram_tensor("wk", [D + 1, D], BF16, kind="ExternalInput").ap()
    wv = nc.dram_tensor("wv", [D + 1, VEXT], BF16, kind="ExternalInput").ap()
    wo = nc.dram_tensor("wo", [D + 1, D], BF16, kind="ExternalInput").ap()
    maskT = nc.dram_tensor("maskT", [TK, TQ], BF16, kind="ExternalInput").ap()
    qres = nc.dram_tensor("qres", [TQ, D], F32, kind="ExternalInput").ap()
    gam = nc.dram_tensor("gam", [1, D], F32, kind="ExternalInput").ap()
    bet = nc.dram_tensor("bet", [1, D], F32, kind="ExternalInput").ap()
    out = nc.dram_tensor("out", [TQ, D], F32, kind="ExternalOutput").ap()

    with TileContext(nc) as tc:
        import contextlib
        with contextlib.ExitStack() as ctx:
            pers = ctx.enter_context(tc.tile_pool(name="pers", bufs=1))

            qhT = pers.tile([128, NFT, TQ], BF16)        # 16 KB/part
            vh_sb = pers.tile([128, NKT, VEXT], BF16)    # 32.5 KB/part
            mk = pers.tile([128, NKT, TQ], BF16)         # 32 KB/part
            attn_sb = pers.tile([128, NPAIR, TQ], BF16)  # 16 KB/part
            ones = pers.tile([1, 512], BF16)             # ones (bias mms)

            # PSUM pools (8 banks): qk 3x2 + pvpair 1x2
            qkps = ctx.enter_context(
                tc.tile_pool(name="qkps", bufs=3, space="PSUM"))
            pvps = ctx.enter_context(
                tc.tile_pool(name="pvps", bufs=1, space="PSUM"))

            # kw outlives the attention pools: its slot is recycled for Wo
            kw = ctx.enter_context(tc.tile_pool(name="kwpool", bufs=1))

            asbuf = ctx.enter_context(contextlib.ExitStack())
            pepool = asbuf.enter_context(tc.tile_pool(name="pepool", bufs=4))
            pmpool = asbuf.enter_context(tc.tile_pool(name="pmpool", bufs=6))
            evpool = asbuf.enter_context(tc.tile_pool(name="evpool", bufs=1))
            kqpool = asbuf.enter_context(tc.tile_pool(name="kqpool", bufs=8))
            khpool = asbuf.enter_context(tc.tile_pool(name="khpool", bufs=2))
            vstack = contextlib.ExitStack()
            vw = vstack.enter_context(
                tc.tile_pool(name="vwpool", bufs=1, side="right"))
            vstage = vstack.enter_context(
                tc.tile_pool(name="vstage", bufs=16, side="right"))

            nc.vector.memset(ones, 1.0)

            # HAM warm-up: K=128 garbage matmuls (uninitialized qhT reads, no
            # input deps) register as sustained PE activity (~3.4us flips the
            # clock gate to 8/8) and bridge the first staging DMAs.
            wtile = qkps.tile([128, 1024], F32, tag="qk", name="warm")
            for _ in range(20):
                nc.tensor.matmul(wtile[0:64, 0:512], qhT[:, 1, 0:64],
                                 qhT[:, 0, 0:512], start=True, stop=True)

            # ---------------- K / V projection emitters ---------------------
            # kT staging is split load/mm: the 8 [128,512] staging DMAs for a
            # quarter are issued well before their matmuls so the PE FIFO
            # never head-of-line blocks on them (v2's dead zones).  Steady-
            # state loads ride the sync ring (keeping the ACT engine queue
            # free of DMA triggers); startup loads ride the gpsimd ring.
            def emit_kproj_load(qtr, eng=None):
                eng = eng if eng is not None else nc.sync
                qs = slice(qtr * 512, (qtr + 1) * 512)
                tiles = []
                for ki in range(NFT):
                    kq_t = kqpool.tile([128, 512], BF16, tag="ktq", name="ktq")
                    eng.dma_start(
                        out=kq_t, in_=kT[ki * 128:(ki + 1) * 128, qs])
                    tiles.append(kq_t)
                return tiles

            def emit_kproj_mm(j, khp, qtr, tiles):
                # khp[:, qtr] = (k @ Wk + bk).T rows j*128.., key qtr slice
                qs = slice(qtr * 512, (qtr + 1) * 512)
                ps_t = qkps.tile([128, 1024], F32, tag="qk", name="qk")
                ps = ps_t[:, 0:512]
                fs = slice(j * 128, (j + 1) * 128)
                for ki in range(NFT):
                    nc.tensor.matmul(ps, wk_m[:, ki, fs], tiles[ki],
                                     start=(ki == 0),
                                     stop=(trivial_bias and ki == NFT - 1))
                if not trivial_bias:
                    nc.tensor.matmul(ps, wk_b[0:1, fs], ones[0:1, 0:512],
                                     start=False, stop=True)
                # khp evac on the ACT engine: DVE is the steady-state
                # bottleneck (mask muls + evac), ACT has ~2us/pass slack
                nc.scalar.copy(khp[:, qs], ps)

            VCH = [(0, 512), (512, 1024), (1024, VEXT)]

            def emit_vproj_load(ti):
                tsl = slice(ti * 128, (ti + 1) * 128)
                vts = []
                for ki in range(NFT):
                    vt = vstage.tile([128, 128], BF16, tag="vT_m", name="vTm")
                    nc.scalar.dma_start(
                        out=vt, in_=vT[ki * 128:(ki + 1) * 128, tsl])
                    vts.append(vt)
                return vts

            def emit_vproj_mm(ti, vts):
                # vh_sb[:, ti, :] = (v @ Wv_ext + bv_ext).T tile ti (128 keys)
                tsl = slice(ti * 128, (ti + 1) * 128)
                for (c0, c1) in VCH:
                    ps_t = qkps.tile([128, 1024], F32, tag="qk", name="qk")
                    ps = ps_t[:, 0:512]
                    n = c1 - c0
                    for ki in range(NFT):
                        nc.tensor.matmul(ps[:, 0:n], vts[ki],
                                         wv_m[:, ki, c0:c1],
                                         start=(ki == 0), stop=False)
                    nc.tensor.matmul(ps[:, 0:n], ones[0:1, 0:128],
                                     wv_b[0:1, c0:c1], start=False, stop=True)
                    if c0 == 0:
                        nc.scalar.copy(vh_sb[:, ti, c0:c1], ps[:, 0:n])
                    else:
                        nc.vector.tensor_copy(vh_sb[:, ti, c0:c1], ps[:, 0:n])

            # --------- startup: kproj pair 0 + Q-proj, interleaved ----------
            # kproj qtr0 needs only wk-h0 + one staged group (2 MB): it runs
            # first, right behind the warm-up; Q-proj chunks follow as their
            # DMA halves land; remaining kproj qtrs fill between them.
            khps = [None] * NPAIR
            khps[0] = khpool.tile([128, TK], BF16, tag="khp", name="khp")
            kq_tiles = {}

            # DMA priority: wq0/qT0 lead the two HWDGE rings so the Q-proj
            # half-A matmuls start ~14us in; kproj staging + wk ride the
            # gpsimd SWDGE ring (wk0, kq0-3), then wv and the mask.
            wk_m = kw.tile([128, NFT, D], BF16, tag="wk_m")

            with tc.tile_pool(name="qppool", bufs=1, side="right") as qp:
                wq_m = qp.tile([128, NFT, D], BF16, tag="wq_m")
                qT_m = qp.tile([128, NFT, TQ], BF16, tag="qT_m")
                nc.sync.dma_start(
                    out=wq_m[:, 0:4, :],
                    in_=wq[0:512, :].rearrange("(k p) f -> p k f", p=128))
                nc.scalar.dma_start(
                    out=qT_m[:, 0:4, :],
                    in_=qT[0:512, :].rearrange("(k p) t -> p k t", p=128))
                nc.sync.dma_start(
                    out=wq_m[:, 4:8, :],
                    in_=wq[512:D, :].rearrange("(k p) f -> p k f", p=128))
                nc.scalar.dma_start(
                    out=qT_m[:, 4:8, :],
                    in_=qT[512:D, :].rearrange("(k p) t -> p k t", p=128))
                nc.gpsimd.dma_start(
                    out=wk_m[:, :, 0:512],
                    in_=wk[0:D, 0:512].rearrange("(k p) f -> p k f", p=128))
                kq_tiles[0] = emit_kproj_load(0, nc.gpsimd)
                nc.sync.dma_start(
                    out=wk_m[:, :, 512:D],
                    in_=wk[0:D, 512:D].rearrange("(k p) f -> p k f", p=128))
                if not trivial_bias:
                    wq_b = qp.tile([1, D], BF16, tag="wq_b")
                    nc.sync.dma_start(out=wq_b, in_=wq[D:D + 1, :])
                    wk_b = kw.tile([1, D], BF16, tag="wk_b")
                    nc.sync.dma_start(out=wk_b, in_=wk[D:D + 1, :])

                # Q-proj as two half-contractions: half A (ki 0-3) runs on
                # the first wq/qT DMA halves, half B accumulates in psum and
                # a DVE add folds the bf16 half-A partials back in.  attn_sb
                # (first written at pass-1 kt2) doubles as half-A scratch.
                qh_a = attn_sb

                def emit_qproj_half(half, c):
                    cs = slice(c * 512, (c + 1) * 512)
                    k0 = 4 * half
                    for fi in range(NFT):
                        fs = slice(fi * 128, (fi + 1) * 128)
                        ps_t = qkps.tile([128, 1024], F32, tag="qk", name="qk")
                        ps = ps_t[:, 0:512]
                        for ki in range(k0, k0 + 4):
                            stop = (ki == k0 + 3) and (half == 0 or trivial_bias)
                            nc.tensor.matmul(ps, wq_m[:, ki, fs], qT_m[:, ki, cs],
                                             start=(ki == k0), stop=stop)
                        if half == 0:
                            if fi % 2 == 0:
                                nc.scalar.copy(qh_a[:, fi, cs], ps)
                            else:
                                nc.vector.tensor_copy(qh_a[:, fi, cs], ps)
                        else:
                            if not trivial_bias:
                                nc.tensor.matmul(ps, wq_b[0:1, fs],
                                                 ones[0:1, 0:512],
                                                 start=False, stop=True)
                            nc.vector.tensor_add(qhT[:, fi, cs], ps,
                                                 qh_a[:, fi, cs])

                emit_qproj_half(0, 0)
                emit_qproj_half(0, 1)
                emit_kproj_mm(0, khps[0], 0, kq_tiles.pop(0))
                kq_tiles[1] = emit_kproj_load(1, nc.gpsimd)
                emit_qproj_half(1, 0)
                emit_kproj_mm(0, khps[0], 1, kq_tiles.pop(1))
                kq_tiles[2] = emit_kproj_load(2, nc.gpsimd)
                emit_qproj_half(1, 1)

            # wv + mask stream behind the staging groups on the gpsimd ring
            wv_m = vw.tile([128, NFT, VEXT], BF16, tag="wv_m")
            wv_b = vw.tile([1, VEXT], BF16, tag="wv_b")
            nc.gpsimd.dma_start(
                out=wv_m, in_=wv[0:D, :].rearrange("(k p) f -> p k f", p=128))
            nc.gpsimd.dma_start(out=wv_b, in_=wv[D:D + 1, :])
            for mc in range(4):
                ts = slice(mc * 512, (mc + 1) * 512)
                nc.gpsimd.dma_start(
                    out=mk[:, 4 * mc:4 * mc + 4, :],
                    in_=maskT[ts, :].rearrange("(t p) q -> p t q", p=128))

            emit_kproj_mm(0, khps[0], 2, kq_tiles.pop(2))
            kq_tiles[3] = emit_kproj_load(3, nc.gpsimd)
            emit_kproj_mm(0, khps[0], 3, kq_tiles.pop(3))

            vloads = {0: emit_vproj_load(0), 1: emit_vproj_load(1)}
            emit_vproj_mm(0, vloads.pop(0))
            vloads[2] = emit_vproj_load(2)
            emit_vproj_mm(1, vloads.pop(1))

            pend_evac = [None]
            pend_norm = [None]
            # kproj group deferred from the previous pass end: (jn, qtr, tiles)
            pend_kmm = [None]

            def run_pass(pi):
                j, qh = pi // 2, pi % 2
                khp = khps[j]
                qsl = slice(qh * 512, (qh + 1) * 512)
                h0sl = slice((2 * j) * 65, (2 * j) * 65 + 65)
                h1sl = slice((2 * j + 1) * 65, (2 * j + 1) * 65 + 65)
                lag = 2 if pi == 0 else 3
                pvpair = pvps.tile([65, 2, 512], F32, tag="pv", name="pv")
                pv0 = pvpair[:, 0, :]
                pv1 = pvpair[:, 1, :]
                work = []

                def emit_pv():
                    pm0, pm1, kt = work.pop(0)
                    nc.tensor.matmul(pv0, vh_sb[:, kt, h0sl], pm0,
                                     start=(kt == 0), stop=(kt == NKT - 1))
                    nc.tensor.matmul(pv1, vh_sb[:, kt, h1sl], pm1,
                                     start=(kt == 0), stop=(kt == NKT - 1))

                qks = {}

                def emit_qk(kt):
                    tsl = slice(kt * 128, (kt + 1) * 128)
                    qk = qkps.tile([128, 1024], F32, tag="qk", name="qk")
                    # concurrent row-tiled QK: h-even rows 0-63, h-odd 64-127
                    nc.tensor.matmul(qk[:, 0:512], khp[0:64, tsl],
                                     qhT[0:64, j, qsl], start=True, stop=True)
                    nc.tensor.matmul(qk[:, 512:1024], khp[64:128, tsl],
                                     qhT[64:128, j, qsl], start=True, stop=True)
                    qks[kt] = qk

                emit_qk(0)
                for kt in range(NKT):
                    # QK one iteration ahead: keeps the exp stream in front
                    # of interleaved kproj/vproj/PV work in the PE FIFO
                    if kt + 1 < NKT:
                        emit_qk(kt + 1)
                    qk = qks.pop(kt)
                    pe = pepool.tile([128, 1024], BF16, tag="pe", name="pe")
                    nc.scalar.activation(pe, qk, AF.Exp)
                    pm0 = pmpool.tile([128, 512], BF16, tag="pm", name="pm")
                    pm1 = pmpool.tile([128, 512], BF16, tag="pm", name="pm")
                    nc.vector.tensor_mul(pm0, pe[:, 0:512], mk[:, kt, qsl])
                    nc.vector.tensor_mul(pm1, pe[:, 512:1024], mk[:, kt, qsl])
                    work.append((pm0, pm1, kt))
                    # software-pipelined evac of the previous pass, two-phase:
                    # rowsum recip at kt2, normalized move-out at kt4
                    if kt == 2 and pend_evac[0] is not None:
                        pend_evac[0]()
                        pend_evac[0] = None
                    if kt == 6 and pend_norm[0] is not None:
                        pend_norm[0]()
                        pend_norm[0] = None
                    # interleaved projection work (fills PE slack)
                    if pi == 0 and kt < 14:
                        if kt < 13:
                            vloads[kt + 3] = emit_vproj_load(kt + 3)
                        emit_vproj_mm(kt + 2, vloads.pop(kt + 2))
                    # deferred kproj group from the previous pass boundary
                    # (staged load fired at that boundary, ~2.5us ago)
                    if kt == 3 and pend_kmm[0] is not None:
                        jn, qtr, tiles = pend_kmm[0]
                        pend_kmm[0] = None
                        emit_kproj_mm(jn, khps[jn], qtr, tiles)
                    # stage the next pair's group consumed at this pass's end
                    if kt == 7 and j < NPAIR - 1:
                        qa = 0 if pi % 2 == 0 else 2
                        kq_tiles[qa] = emit_kproj_load(qa)
                    tgt = min(lag, NKT - 1 - kt)
                    while len(work) > tgt:
                        emit_pv()
                while work:
                    emit_pv()
                # pass tail: kproj group A fills the PE while the exp tail
                # drains (keeps HAM warm across the boundary); group B's
                # staging fires now and its matmuls run at kt3 of the next
                # pass, bridging the boundary.
                if j < NPAIR - 1:
                    qa = 0 if pi % 2 == 0 else 2
                    qb = qa + 1
                    if khps[j + 1] is None:
                        khps[j + 1] = khpool.tile([128, TK], BF16,
                                                  tag="khp", name="khp")
                    emit_kproj_mm(j + 1, khps[j + 1], qa, kq_tiles.pop(qa))
                    kq_tiles[qb] = emit_kproj_load(qb)
                    pend_kmm[0] = (j + 1, qb, kq_tiles.pop(qb))

                def evac():
                    # pvpair-critical phase: rowsums + raw (unnormalized) PV
                    # out of the psum; bf16 attn is scale-invariant-safe
                    rs = evpool.tile([1, 2, 512], F32, tag="rs", name="rs")
                    nc.vector.tensor_copy(rs, pvpair[64:65, :, :])
                    nc.vector.tensor_copy(attn_sb[0:64, j, qsl],
                                          pvpair[0:64, 0, :])
                    nc.vector.tensor_copy(attn_sb[64:128, j, qsl],
                                          pvpair[0:64, 1, :])

                    def norm():
                        nc.vector.reciprocal_approx_fast(rs, rs)
                        rrb = evpool.tile([128, 2, 512], F32, tag="rrb",
                                          name="rrb")
                        nc.gpsimd.partition_broadcast(rrb, rs)
                        # normalize on gpsimd: DVE is the steady-state
                        # bottleneck and these are off the critical path
                        for hh in (0, 1):
                            sl = attn_sb[64 * hh:64 * hh + 64, j, qsl]
                            nc.gpsimd.tensor_mul(
                                sl, sl, rrb[64 * hh:64 * hh + 64, hh, :])
                    pend_norm[0] = norm
                pend_evac[0] = evac

            for pi in range(2 * NPAIR):
                run_pass(pi)
                if pi == 0:
                    vstack.close()
                if pi == 13:
                    # Wk's last reader (pair-7 kproj) has been emitted; its
                    # kw slot is recycled for Wo (WAR dep, DMA runs ~pass 14)
                    wo_m = kw.tile([128, NFT, D], BF16, tag="wk_m")
                    nc.sync.dma_start(
                        out=wo_m[:, :, 0:512],
                        in_=wo[0:D, 0:512].rearrange("(k p) f -> p k f", p=128))
                    nc.sync.dma_start(
                        out=wo_m[:, :, 512:1024],
                        in_=wo[0:D, 512:1024].rearrange("(k p) f -> p k f", p=128))
                    if not trivial_bias:
                        wo_b = kw.tile([1, D], BF16, tag="wk_b")
                        nc.sync.dma_start(out=wo_b, in_=wo[D:D + 1, :])
            pend_evac[0]()
            pend_evac[0] = None
            pend_norm[0]()
            pend_norm[0] = None
            # free attention-phase SBUF pools (space reused by phase C pools;
            # WAR deps keep the pass-15 tail and evac correct)
            asbuf.close()

            # ------------ phase C: out-proj + residual + LN -----------------
            # O-proj psum comes from the qk pool; emitted right after pass 15
            # so qh=0 projection overlaps the last evac / pass tail.
            with tc.tile_pool(name="cq", bufs=2) as cq, \
                 tc.tile_pool(name="cl", bufs=2) as cl:

                eps_t = cl.tile([128, 1], F32, tag="eps")
                nc.vector.memset(eps_t, 1e-5)
                if not trivial_affine:
                    gam_r = cl.tile([1, D], F32, tag="gam_r")
                    bet_r = cl.tile([1, D], F32, tag="bet_r")
                    nc.sync.dma_start(out=gam_r, in_=gam)
                    nc.sync.dma_start(out=bet_r, in_=bet)
                    gam_b = cl.tile([128, D], F32, tag="gam_b")
                    bet_b = cl.tile([128, D], F32, tag="bet_b")
                    nc.gpsimd.partition_broadcast(gam_b, gam_r)
                    nc.gpsimd.partition_broadcast(bet_b, bet_r)

                for qt in range(NFT):
                    qts = slice(qt * 128, (qt + 1) * 128)
                    qres_t = cq.tile([128, D], F32, tag="qres")
                    nc.scalar.dma_start(out=qres_t, in_=qres[qts, :])
                    ps = qkps.tile([128, 1024], F32, tag="qk", name="oproj")
                    for c in range(2):
                        cs = slice(c * 512, (c + 1) * 512)
                        for ki in range(NFT):
                            nc.tensor.matmul(ps[:, cs], attn_sb[:, ki, qts],
                                             wo_m[:, ki, cs],
                                             start=(ki == 0),
                                             stop=(trivial_bias and ki == NFT - 1))
                        if not trivial_bias:
                            nc.tensor.matmul(ps[:, cs], ones[0:1, 0:128],
                                             wo_b[0:1, cs], start=False, stop=True)
                    x_sb = cq.tile([128, D], F32, tag="x_sb")
                    nc.vector.tensor_add(x_sb, ps, qres_t)

                    stats = cl.tile([128, 2, 6], F32, tag="stats")
                    nc.vector.bn_stats(stats[:, 0, :], x_sb[:, 0:512])
                    nc.vector.bn_stats(stats[:, 1, :], x_sb[:, 512:1024])
                    mv = cl.tile([128, 2], F32, tag="mv")
                    nc.vector.bn_aggr(mv, stats)
                    sq = cl.tile([128, 1], F32, tag="sq")
                    nc.scalar.activation(sq, mv[:, 1:2], AF.Sqrt, bias=eps_t)
                    rstd = cl.tile([128, 1], F32, tag="rstd")
                    nc.vector.reciprocal(rstd, sq)
                    xo = cl.tile([128, D], F32, tag="xo")
                    nc.vector.tensor_scalar(xo, x_sb, mv[:, 0:1], rstd,
                                            op0=mybir.AluOpType.subtract,
                                            op1=mybir.AluOpType.mult)
                    if not trivial_affine:
                        nc.vector.tensor_mul(xo, xo, gam_b)
                        nc.vector.tensor_add(xo, xo, bet_b)
                    nc.sync.dma_start(out=out[qts, :], in_=xo)
    return nc


def _prep_core_inputs(inputs, b, qh):
    """Build the per-core input map (host-side layout prep only)."""
    import ml_dtypes
    bf = ml_dtypes.bfloat16
    q = np.asarray(inputs["q"], np.float32)
    k = np.asarray(inputs["k"], np.float32)
    v = np.asarray(inputs["v"], np.float32)
    mask = np.asarray(inputs["attn_mask"])
    Wq, bq = np.asarray(inputs["Wq"], np.float32), np.asarray(inputs["bq"], np.float32)
    Wk, bk = np.asarray(inputs["Wk"], np.float32), np.asarray(inputs["bk"], np.float32)
    Wv, bv = np.asarray(inputs["Wv"], np.float32), np.asarray(inputs["bv"], np.float32)
    Wo, bo = np.asarray(inputs["Wo"], np.float32), np.asarray(inputs["bo"], np.float32)
    gamma, beta = np.asarray(inputs["gamma"], np.float32), np.asarray(inputs["beta"], np.float32)

    qs = slice(qh * TQ, (qh + 1) * TQ)
    qb = q[b, qs, :]                       # [TQ, D]

    def ext_T(x_t):  # [D, N] -> [D+1, N] with ones row
        return np.concatenate([x_t, np.ones((1, x_t.shape[1]), np.float32)], axis=0)

    def ext_W(W, bias):  # [D, N] -> [D+1, N] with bias row
        return np.concatenate([W, bias[None, :]], axis=0)

    # Wv extended with per-head ones column: col h*65+64 gets bias 1, weights 0
    Wv_ext = np.zeros((D, VEXT), np.float32)
    bv_ext = np.zeros((VEXT,), np.float32)
    for h in range(H):
        Wv_ext[:, h * 65:h * 65 + 64] = Wv[:, h * 64:(h + 1) * 64]
        bv_ext[h * 65:h * 65 + 64] = bv[h * 64:(h + 1) * 64]
        bv_ext[h * 65 + 64] = 1.0

    return {
        "qT": ext_T(qb.T.copy()).astype(bf),
        "kT": ext_T(k[b].T.copy()).astype(bf),
        "vT": ext_T(v[b].T.copy()).astype(bf),
        "wq": ext_W(Wq, bq).astype(bf),
        "wk": ext_W(Wk, bk).astype(bf),
        "wv": ext_W(Wv_ext, bv_ext).astype(bf),
        "wo": ext_W(Wo, bo).astype(bf),
        "maskT": np.ascontiguousarray(mask[b, qs, :].T).astype(bf),
        "qres": np.ascontiguousarray(qb),
        "gam": gamma[None, :].copy(),
        "bet": beta[None, :].copy(),
    }


def kernel(**inputs) -> np.ndarray:
    global _LAST_RESULTS
    trivial_affine = (np.all(np.asarray(inputs["gamma"]) == 1.0)
                      and np.all(np.asarray(inputs["beta"]) == 0.0))
    trivial_bias = all(
        np.all(np.asarray(inputs[k]) == 0.0) for k in ("bq", "bk", "bv", "bo"))
    nc = bacc.Bacc("TRN2", debug=False, num_devices=NCORES)
    build_program(nc, trivial_affine=trivial_affine, trivial_bias=trivial_bias)
    nc.finalize()

    ncores_run = int(os.environ.get("KERNEL_NCORES", str(NCORES)))
    in_maps = [_prep_core_inputs(inputs, c // 2, c % 2) for c in range(NCORES)]
    trace = bool(int(os.environ.get("KERNEL_TRACE", "0")))
    res = run_bass_kernel_spmd(nc, in_maps[:ncores_run],
                               core_ids=list(range(ncores_run)), trace=trace)
    _LAST_RESULTS = {"exec_time_ns": res.exec_time_ns,
                     "profile_json": res.profile_json,
                     "res": res}

    out = np.empty((B, T, D), np.float32)
    for c in range(NCORES):
        b, qh = c // 2, c % 2
        out[b, qh * TQ:(qh + 1) * TQ, :] = res.results[c % ncores_run]["out"]
    return out


# revision 77
# speedup vs baseline: 1.0435x; 1.0435x over previous
"""Trainium2 Bass kernel for nn_MultiHeadAttention_76587856823057.

Sharding: (batch, query-half) -> 8 cores, zero collectives.
Per core: b fixed, queries TQ=1024 (half of T), all H=16 heads, all TK=2048 keys.

v4 design notes (delta over v2, which measured 620us; v4 measures ~533-542us):
 - warm-up matmuls are K=128 (full contraction rows): v2's K=1 warm-ups never
   registered as PE activity for the HAM clock gate, so the first ~32us ran
   at 1.2 GHz.  They read uninitialized qhT (no input deps, WAR only).
 - Q-proj runs as two half-contractions (ki 0-3, then 4-7 + DVE add folding
   the bf16 half-A partials back in) so it starts on the FIRST halves of the
   wq/qT DMAs (~14us); attn_sb doubles as the half-A scratch (unused until
   pass-1 kt2).  Startup is HBM-bandwidth-bound (~13MB critical set), so the
   rings are prioritized: wq/qT lead the two HWDGE rings, wk + pair-0 kT
   staging + wv + mask ride the gpsimd SWDGE ring.
 - kT staging is split load/mm with loads fired ~8 kt-iterations ahead on
   the sync ring (v2 issued them just-in-time on the PE-consuming path and
   head-of-line blocked all engines ~2x1.7us per pass).
 - kproj matmul groups run at the pass TAIL + kt3 of the next pass: they
   fill the PE idle while the last ~4 exps of the pass drain, which also
   stops the HAM clock gate re-throttling at every pass boundary.
 - pv0/pv1 merged into one [65, 2, 512] psum tile -> evac: one [1,1024]
   rowsum copy + approx-reciprocal, raw (unnormalized, scale-safe) bf16 PV
   move-out at kt2 frees the psum, one full-width gpsimd broadcast and
   in-place normalize muls at kt6.
 - khp psum evacuation on the scalar engine (DVE is the steady-state
   bottleneck: mask muls + evac ~18.8us/pass vs ACT exp 16.8us/pass).
 - Wo reuses the wk SBUF slot (allocated after pair-7 kproj, pass 13);
   phase C allocates its O-proj psum from the qk pool and is emitted right
   after pass 15 so the qh=0 output projection overlaps the tail.
 - everything else as v2: softmax shift-invariance (no global max), one mask
   multiply after exp, PE row-tiled QK (2 heads concurrent), rowsums via a
   ones-column in the V projection (M=65 PV), bf16 compute, biases via
   ones-row K=1 matmuls, software-pipelined evacuation.
 - measured engine budgets/pass (steady): DVE ~18.8us (91%), PE ~18us (87%),
   ACT exp 16x1.05us (81%); pass cadence ~20.7us.  Known remaining headroom:
   K/V-proj duplication across the q-half core pairs (~60us PE, needs a
   pair collective), exp [128,2048] batching (needs 2 more PSUM banks).

Self-contained: hardcodes all shapes; no sibling imports.
"""

import os
import numpy as np

import concourse.bass as bass
from concourse import bacc
import concourse.mybir as mybir
from concourse.tile import TileContext
from concourse.bass_utils import run_bass_kernel_spmd

F32 = mybir.dt.float32
BF16 = mybir.dt.bfloat16
AF = mybir.ActivationFunctionType

B, T, D, H, DK = 4, 2048, 1024, 16, 64
TQ = T // 2          # queries per core
TK = T               # keys per core
NCORES = 8
NPAIR = H // 2       # 8 head pairs
NFT = D // 128       # 8 feature tiles
NKT = TK // 128      # 16 key tiles
VEXT = H * (DK + 1)  # 1040: per-head [64 v-cols + ones col]

_LAST_RESULTS = {}


def build_program(nc: bass.Bass, trivial_affine: bool = False,
                  trivial_bias: bool = False):
    # ---- per-core DRAM I/O ----
    qT = nc.dram_tensor("qT", [D + 1, TQ], BF16, kind="ExternalInput").ap()
    kT = nc.dram_tensor("kT", [D + 1, TK], BF16, kind="ExternalInput").ap()
    vT = nc.dram_tensor("vT", [D + 1, TK], BF16, kind="ExternalInput").ap()
    wq = nc.dram_tensor("wq", [D + 1, D], BF16, kind="ExternalInput").ap()
    wk = nc.dram_tensor("wk", [D + 1, D], BF16, kind="ExternalInput").ap()
    wv = nc.dram_tensor("wv", [D + 1, VEXT], BF16, kind="ExternalInput").ap()
    wo = nc.dram_tensor("wo", [D + 1, D], BF16, kind="ExternalInput").ap()
    maskT = nc.dram_tensor("maskT", [TK, TQ], BF16, kind="ExternalInput").ap()
    qres = nc.dram_tensor("qres", [TQ, D], BF16, kind="ExternalInput").ap()
    ident = nc.dram_tensor("ident", [128, 128], BF16, kind="ExternalInput").ap()
    gam = nc.dram_tensor("gam", [1, D], F32, kind="ExternalInput").ap()
    bet = nc.dram_tensor("bet", [1, D], F32, kind="ExternalInput").ap()
    out = nc.dram_tensor("out", [TQ, D], F32, kind="ExternalOutput").ap()

    with TileContext(nc) as tc:
        import contextlib
        with contextlib.ExitStack() as ctx:
            pers = ctx.enter_context(tc.tile_pool(name="pers", bufs=1))

            qhT = pers.tile([128, NFT, TQ], BF16)        # 16 KB/part
            vh_sb = pers.tile([128, NKT, VEXT], BF16)    # 32.5 KB/part
            mk = pers.tile([128, NKT, TQ], BF16)         # 32 KB/part
            attn_sb = pers.tile([128, NPAIR, TQ], BF16)  # 16 KB/part
            ones = pers.tile([1, 512], BF16)             # ones (bias mms)
            ident_sb = pers.tile([128, 128], BF16)       # residual-add mm
            nc.sync.dma_start(out=ident_sb, in_=ident)

            # PSUM pools (8 banks): qk 3x2 + pvpair 1x2
            qkps = ctx.enter_context(
                tc.tile_pool(name="qkps", bufs=3, space="PSUM"))
            pvps = ctx.enter_context(
                tc.tile_pool(name="pvps", bufs=1, space="PSUM"))

            # kw outlives the attention pools: its slot is recycled for Wo
            kw = ctx.enter_context(tc.tile_pool(name="kwpool", bufs=1))

            asbuf = ctx.enter_context(contextlib.ExitStack())
            pepool = asbuf.enter_context(tc.tile_pool(name="pepool", bufs=4))
            pmpool = asbuf.enter_context(tc.tile_pool(name="pmpool", bufs=6))
            evpool = asbuf.enter_context(tc.tile_pool(name="evpool", bufs=1))
            kqpool = asbuf.enter_context(tc.tile_pool(name="kqpool", bufs=8))
            khpool = asbuf.enter_context(tc.tile_pool(name="khpool", bufs=2))
            vstack = contextlib.ExitStack()
            vw = vstack.enter_context(
                tc.tile_pool(name="vwpool", bufs=1, side="right"))
            vstage = vstack.enter_context(
                tc.tile_pool(name="vstage", bufs=16, side="right"))

            nc.vector.memset(ones, 1.0)

            # HAM warm-up: K=128 garbage matmuls (uninitialized qhT reads, no
            # input deps) register as sustained PE activity (~3.4us flips the
            # clock gate to 8/8) and bridge the first staging DMAs.
            wtile = qkps.tile([128, 1024], F32, tag="qk", name="warm")
            for _ in range(20):
                nc.tensor.matmul(wtile[0:64, 0:512], qhT[:, 1, 0:64],
                                 qhT[:, 0, 0:512], start=True, stop=True)

            # ---------------- K / V projection emitters ---------------------
            # kT staging is split load/mm: the 8 [128,512] staging DMAs for a
            # quarter are issued well before their matmuls so the PE FIFO
            # never head-of-line blocks on them (v2's dead zones).  Steady-
            # state loads ride the sync ring (keeping the ACT engine queue
            # free of DMA triggers); startup loads ride the gpsimd ring.
            def emit_kproj_load(qtr, eng=None):
                eng = eng if eng is not None else nc.sync
                qs = slice(qtr * 512, (qtr + 1) * 512)
                tiles = []
                for ki in range(NFT):
                    kq_t = kqpool.tile([128, 512], BF16, tag="ktq", name="ktq")
                    eng.dma_start(
                        out=kq_t, in_=kT[ki * 128:(ki + 1) * 128, qs])
                    tiles.append(kq_t)
                return tiles

            def emit_kproj_mm(j, khp, qtr, tiles):
                # khp[:, qtr] = (k @ Wk + bk).T rows j*128.., key qtr slice
                qs = slice(qtr * 512, (qtr + 1) * 512)
                ps_t = qkps.tile([128, 1024], F32, tag="qk", name="qk")
                ps = ps_t[:, 0:512]
                fs = slice(j * 128, (j + 1) * 128)
                for ki in range(NFT):
                    nc.tensor.matmul(ps, wk_m[:, ki, fs], tiles[ki],
                                     start=(ki == 0),
                                     stop=(trivial_bias and ki == NFT - 1))
                if not trivial_bias:
                    nc.tensor.matmul(ps, wk_b[0:1, fs], ones[0:1, 0:512],
                                     start=False, stop=True)
                # khp evac on the ACT engine: DVE is the steady-state
                # bottleneck (mask muls + evac), ACT has ~2us/pass slack
                nc.scalar.copy(khp[:, qs], ps)

            VCH = [(0, 512), (512, 1024), (1024, VEXT)]

            def emit_vproj_load(ti):
                tsl = slice(ti * 128, (ti + 1) * 128)
                vts = []
                for ki in range(NFT):
                    vt = vstage.tile([128, 128], BF16, tag="vT_m", name="vTm")
                    nc.scalar.dma_start(
                        out=vt, in_=vT[ki * 128:(ki + 1) * 128, tsl])
                    vts.append(vt)
                return vts

            def emit_vproj_mm(ti, vts):
                # vh_sb[:, ti, :] = (v @ Wv_ext + bv_ext).T tile ti (128 keys)
                tsl = slice(ti * 128, (ti + 1) * 128)
                for (c0, c1) in VCH:
                    ps_t = qkps.tile([128, 1024], F32, tag="qk", name="qk")
                    ps = ps_t[:, 0:512]
                    n = c1 - c0
                    for ki in range(NFT):
                        nc.tensor.matmul(ps[:, 0:n], vts[ki],
                                         wv_m[:, ki, c0:c1],
                                         start=(ki == 0), stop=False)
                    nc.tensor.matmul(ps[:, 0:n], ones[0:1, 0:128],
                                     wv_b[0:1, c0:c1], start=False, stop=True)
                    if c0 == 0:
                        nc.scalar.copy(vh_sb[:, ti, c0:c1], ps[:, 0:n])
                    else:
                        nc.vector.tensor_copy(vh_sb[:, ti, c0:c1], ps[:, 0:n])

            # --------- startup: kproj pair 0 + Q-proj, interleaved ----------
            # kproj qtr0 needs only wk-h0 + one staged group (2 MB): it runs
            # first, right behind the warm-up; Q-proj chunks follow as their
            # DMA halves land; remaining kproj qtrs fill between them.
            khps = [None] * NPAIR
            khps[0] = khpool.tile([128, TK], BF16, tag="khp", name="khp")
            kq_tiles = {}

            # DMA priority: wq0/qT0 lead the two HWDGE rings so the Q-proj
            # half-A matmuls start ~14us in; kproj staging + wk ride the
            # gpsimd SWDGE ring (wk0, kq0-3), then wv and the mask.
            wk_m = kw.tile([128, NFT, D], BF16, tag="wk_m")

            with tc.tile_pool(name="qppool", bufs=1, side="right") as qp:
                wq_m = qp.tile([128, NFT, D], BF16, tag="wq_m")
                qT_m = qp.tile([128, NFT, TQ], BF16, tag="qT_m")
                nc.sync.dma_start(
                    out=wq_m[:, 0:4, :],
                    in_=wq[0:512, :].rearrange("(k p) f -> p k f", p=128))
                nc.scalar.dma_start(
                    out=qT_m[:, 0:4, :],
                    in_=qT[0:512, :].rearrange("(k p) t -> p k t", p=128))
                nc.sync.dma_start(
                    out=wq_m[:, 4:8, :],
                    in_=wq[512:D, :].rearrange("(k p) f -> p k f", p=128))
                nc.scalar.dma_start(
                    out=qT_m[:, 4:8, :],
                    in_=qT[512:D, :].rearrange("(k p) t -> p k t", p=128))
                nc.gpsimd.dma_start(
                    out=wk_m[:, :, 0:512],
                    in_=wk[0:D, 0:512].rearrange("(k p) f -> p k f", p=128))
                kq_tiles[0] = emit_kproj_load(0, nc.gpsimd)
                nc.sync.dma_start(
                    out=wk_m[:, :, 512:D],
                    in_=wk[0:D, 512:D].rearrange("(k p) f -> p k f", p=128))
                if not trivial_bias:
                    wq_b = qp.tile([1, D], BF16, tag="wq_b")
                    nc.sync.dma_start(out=wq_b, in_=wq[D:D + 1, :])
                    wk_b = kw.tile([1, D], BF16, tag="wk_b")
                    nc.sync.dma_start(out=wk_b, in_=wk[D:D + 1, :])

                # Q-proj as two half-contractions: half A (ki 0-3) runs on
                # the first wq/qT DMA halves, half B accumulates in psum and
                # a DVE add folds the bf16 half-A partials back in.  attn_sb
                # (first written at pass-1 kt2) doubles as half-A scratch.
                qh_a = attn_sb

                def emit_qproj_half(half, c):
                    cs = slice(c * 512, (c + 1) * 512)
                    k0 = 4 * half
                    for fi in range(NFT):
                        fs = slice(fi * 128, (fi + 1) * 128)
                        ps_t = qkps.tile([128, 1024], F32, tag="qk", name="qk")
                        ps = ps_t[:, 0:512]
                        for ki in range(k0, k0 + 4):
                            stop = (ki == k0 + 3) and (half == 0 or trivial_bias)
                            nc.tensor.matmul(ps, wq_m[:, ki, fs], qT_m[:, ki, cs],
                                             start=(ki == k0), stop=stop)
                        if half == 0:
                            if fi % 2 == 0:
                                nc.scalar.copy(qh_a[:, fi, cs], ps)
                            else:
                                nc.vector.tensor_copy(qh_a[:, fi, cs], ps)
                        else:
                            if not trivial_bias:
                                nc.tensor.matmul(ps, wq_b[0:1, fs],
                                                 ones[0:1, 0:512],
                                                 start=False, stop=True)
                            nc.vector.tensor_add(qhT[:, fi, cs], ps,
                                                 qh_a[:, fi, cs])

                emit_qproj_half(0, 0)
                emit_qproj_half(0, 1)
                emit_kproj_mm(0, khps[0], 0, kq_tiles.pop(0))
                kq_tiles[1] = emit_kproj_load(1, nc.gpsimd)
                emit_qproj_half(1, 0)
                emit_kproj_mm(0, khps[0], 1, kq_tiles.pop(1))
                kq_tiles[2] = emit_kproj_load(2, nc.gpsimd)
                emit_qproj_half(1, 1)

            # wv + mask stream behind the staging groups on the gpsimd ring
            wv_m = vw.tile([128, NFT, VEXT], BF16, tag="wv_m")
            wv_b = vw.tile([1, VEXT], BF16, tag="wv_b")
            nc.gpsimd.dma_start(
                out=wv_m, in_=wv[0:D, :].rearrange("(k p) f -> p k f", p=128))
            nc.gpsimd.dma_start(out=wv_b, in_=wv[D:D + 1, :])
            for mc in range(4):
                ts = slice(mc * 512, (mc + 1) * 512)
                nc.gpsimd.dma_start(
                    out=mk[:, 4 * mc:4 * mc + 4, :],
                    in_=maskT[ts, :].rearrange("(t p) q -> p t q", p=128))

            emit_kproj_mm(0, khps[0], 2, kq_tiles.pop(2))
            kq_tiles[3] = emit_kproj_load(3, nc.gpsimd)
            emit_kproj_mm(0, khps[0], 3, kq_tiles.pop(3))

            vloads = {0: emit_vproj_load(0), 1: emit_vproj_load(1)}
            emit_vproj_mm(0, vloads.pop(0))
            vloads[2] = emit_vproj_load(2)
            emit_vproj_mm(1, vloads.pop(1))

            pend_evac = [None]
            pend_norm = [None]
            # kproj group deferred from the previous pass end: (jn, qtr, tiles)
            pend_kmm = [None]

            def run_pass(pi):
                j, qh = pi // 2, pi % 2
                khp = khps[j]
                qsl = slice(qh * 512, (qh + 1) * 512)
                h0sl = slice((2 * j) * 65, (2 * j) * 65 + 65)
                h1sl = slice((2 * j + 1) * 65, (2 * j + 1) * 65 + 65)
                lag = 2 if pi == 0 else 3
                pvpair = pvps.tile([65, 2, 512], F32, tag="pv", name="pv")
                pv0 = pvpair[:, 0, :]
                pv1 = pvpair[:, 1, :]
                work = []

                def emit_pv():
                    pm0, pm1, kt = work.pop(0)
                    nc.tensor.matmul(pv0, vh_sb[:, kt, h0sl], pm0,
                                     start=(kt == 0), stop=(kt == NKT - 1))
                    nc.tensor.matmul(pv1, vh_sb[:, kt, h1sl], pm1,
                                     start=(kt == 0), stop=(kt == NKT - 1))

                qks = {}

                def emit_qk(kt):
                    tsl = slice(kt * 128, (kt + 1) * 128)
                    qk = qkps.tile([128, 1024], F32, tag="qk", name="qk")
                    # concurrent row-tiled QK: h-even rows 0-63, h-odd 64-127
                    nc.tensor.matmul(qk[:, 0:512], khp[0:64, tsl],
                                     qhT[0:64, j, qsl], start=True, stop=True)
                    nc.tensor.matmul(qk[:, 512:1024], khp[64:128, tsl],
                                     qhT[64:128, j, qsl], start=True, stop=True)
                    qks[kt] = qk

                emit_qk(0)
                for kt in range(NKT):
                    # QK one iteration ahead: keeps the exp stream in front
                    # of interleaved kproj/vproj/PV work in the PE FIFO
                    if kt + 1 < NKT:
                        emit_qk(kt + 1)
                    qk = qks.pop(kt)
                    pe = pepool.tile([128, 1024], BF16, tag="pe", name="pe")
                    nc.scalar.activation(pe, qk, AF.Exp)
                    pm0 = pmpool.tile([128, 512], BF16, tag="pm", name="pm")
                    pm1 = pmpool.tile([128, 512], BF16, tag="pm", name="pm")
                    nc.vector.tensor_mul(pm0, pe[:, 0:512], mk[:, kt, qsl])
                    nc.vector.tensor_mul(pm1, pe[:, 512:1024], mk[:, kt, qsl])
                    work.append((pm0, pm1, kt))
                    # software-pipelined evac of the previous pass, two-phase:
                    # rowsum recip at kt2, normalized move-out at kt4
                    if kt == 2 and pend_evac[0] is not None:
                        pend_evac[0]()
                        pend_evac[0] = None
                    if kt == 6 and pend_norm[0] is not None:
                        pend_norm[0]()
                        pend_norm[0] = None
                    # interleaved projection work (fills PE slack)
                    if pi == 0 and kt < 14:
                        if kt < 13:
                            vloads[kt + 3] = emit_vproj_load(kt + 3)
                        emit_vproj_mm(kt + 2, vloads.pop(kt + 2))
                    # deferred kproj group from the previous pass boundary
                    # (staged load fired at that boundary, ~2.5us ago)
                    if kt == 3 and pend_kmm[0] is not None:
                        jn, qtr, tiles = pend_kmm[0]
                        pend_kmm[0] = None
                        emit_kproj_mm(jn, khps[jn], qtr, tiles)
                    # stage the next pair's group consumed at this pass's end
                    if kt == 7 and j < NPAIR - 1:
                        qa = 0 if pi % 2 == 0 else 2
                        kq_tiles[qa] = emit_kproj_load(qa)
                    tgt = min(lag, NKT - 1 - kt)
                    while len(work) > tgt:
                        emit_pv()
                while work:
                    emit_pv()
                # pass tail: kproj group A fills the PE while the exp tail
                # drains (keeps HAM warm across the boundary); group B's
                # staging fires now and its matmuls run at kt3 of the next
                # pass, bridging the boundary.
                if j < NPAIR - 1:
                    qa = 0 if pi % 2 == 0 else 2
                    qb = qa + 1
                    if khps[j + 1] is None:
                        khps[j + 1] = khpool.tile([128, TK], BF16,
                                                  tag="khp", name="khp")
                    emit_kproj_mm(j + 1, khps[j + 1], qa, kq_tiles.pop(qa))
                    kq_tiles[qb] = emit_kproj_load(qb)
                    pend_kmm[0] = (j + 1, qb, kq_tiles.pop(qb))

                def evac():
                    # pvpair-critical phase: rowsums + raw (unnormalized) PV
                    # out of the psum; bf16 attn is scale-invariant-safe
                    rs = evpool.tile([1, 2, 512], F32, tag="rs", name="rs")
                    nc.vector.tensor_copy(rs, pvpair[64:65, :, :])
                    nc.vector.tensor_copy(attn_sb[0:64, j, qsl],
                                          pvpair[0:64, 0, :])
                    nc.vector.tensor_copy(attn_sb[64:128, j, qsl],
                                          pvpair[0:64, 1, :])

                    def norm():
                        nc.vector.reciprocal_approx_fast(rs, rs)
                        rrb = evpool.tile([128, 2, 512], F32, tag="rrb",
                                          name="rrb")
                        nc.gpsimd.partition_broadcast(rrb, rs)
                        for hh in (0, 1):
                            sl = attn_sb[64 * hh:64 * hh + 64, j, qsl]
                            nc.vector.tensor_mul(
                                sl, sl, rrb[64 * hh:64 * hh + 64, hh, :])
                    pend_norm[0] = norm
                pend_evac[0] = evac

            for pi in range(2 * NPAIR):
                run_pass(pi)
                if pi == 0:
                    vstack.close()
                if pi == 13:
                    # Wk's last reader (pair-7 kproj) has been emitted; its
                    # kw slot is recycled for Wo (WAR dep, DMA runs ~pass 14)
                    wo_m = kw.tile([128, NFT, D], BF16, tag="wk_m")
                    nc.sync.dma_start(
                        out=wo_m[:, :, 0:512],
                        in_=wo[0:D, 0:512].rearrange("(k p) f -> p k f", p=128))
                    nc.sync.dma_start(
                        out=wo_m[:, :, 512:1024],
                        in_=wo[0:D, 512:1024].rearrange("(k p) f -> p k f", p=128))
                    if not trivial_bias:
                        wo_b = kw.tile([1, D], BF16, tag="wk_b")
                        nc.sync.dma_start(out=wo_b, in_=wo[D:D + 1, :])
            pend_evac[0]()
            pend_evac[0] = None
            pend_norm[0]()
            pend_norm[0] = None
            # free attention-phase SBUF pools (space reused by phase C pools;
            # WAR deps keep the pass-15 tail and evac correct)
            asbuf.close()

            # ------------ phase C: out-proj + residual + LN -----------------
            # O-proj psum comes from the qk pool; emitted right after pass 15
            # so qh=0 projection overlaps the last evac / pass tail.
            with tc.tile_pool(name="cq", bufs=2) as cq, \
                 tc.tile_pool(name="cl", bufs=2) as cl:

                eps_t = cl.tile([128, 1], F32, tag="eps")
                nc.vector.memset(eps_t, 1e-5)
                if not trivial_affine:
                    gam_r = cl.tile([1, D], F32, tag="gam_r")
                    bet_r = cl.tile([1, D], F32, tag="bet_r")
                    nc.sync.dma_start(out=gam_r, in_=gam)
                    nc.sync.dma_start(out=bet_r, in_=bet)
                    gam_b = cl.tile([128, D], F32, tag="gam_b")
                    bet_b = cl.tile([128, D], F32, tag="bet_b")
                    nc.gpsimd.partition_broadcast(gam_b, gam_r)
                    nc.gpsimd.partition_broadcast(bet_b, bet_r)

                for qt in range(NFT):
                    qts = slice(qt * 128, (qt + 1) * 128)
                    qres_t = cq.tile([128, D], BF16, tag="qres")
                    nc.scalar.dma_start(out=qres_t, in_=qres[qts, :])
                    ps = qkps.tile([128, 1024], F32, tag="qk", name="oproj")
                    for c in range(2):
                        cs = slice(c * 512, (c + 1) * 512)
                        for ki in range(NFT):
                            nc.tensor.matmul(ps[:, cs], attn_sb[:, ki, qts],
                                             wo_m[:, ki, cs],
                                             start=(ki == 0), stop=False)
                        if not trivial_bias:
                            nc.tensor.matmul(ps[:, cs], ones[0:1, 0:128],
                                             wo_b[0:1, cs], start=False,
                                             stop=False)
                        # residual folded into the psum via identity matmul
                        # (frees the DVE add in the DVE-bound tail)
                        nc.tensor.matmul(ps[:, cs], ident_sb, qres_t[:, cs],
                                         start=False, stop=True)

                    stats = cl.tile([128, 2, 6], F32, tag="stats")
                    nc.vector.bn_stats(stats[:, 0, :], ps[:, 0:512])
                    nc.vector.bn_stats(stats[:, 1, :], ps[:, 512:1024])
                    mv = cl.tile([128, 2], F32, tag="mv")
                    nc.vector.bn_aggr(mv, stats)
                    sq = cl.tile([128, 1], F32, tag="sq")
                    nc.scalar.activation(sq, mv[:, 1:2], AF.Sqrt, bias=eps_t)
                    rstd = cl.tile([128, 1], F32, tag="rstd")
                    nc.vector.reciprocal(rstd, sq)
                    xo = cl.tile([128, D], F32, tag="xo")
                    nc.vector.tensor_scalar(xo, ps, mv[:, 0:1], rstd,
                                            op0=mybir.AluOpType.subtract,
                                            op1=mybir.AluOpType.mult)
                    if not trivial_affine:
                        nc.vector.tensor_mul(xo, xo, gam_b)
                        nc.vector.tensor_add(xo, xo, bet_b)
                    nc.sync.dma_start(out=out[qts, :], in_=xo)
    return nc


def _prep_core_inputs(inputs, b, qh):
    """Build the per-core input map (host-side layout prep only)."""
    import ml_dtypes
    bf = ml_dtypes.bfloat16
    q = np.asarray(inputs["q"], np.float32)
    k = np.asarray(inputs["k"], np.float32)
    v = np.asarray(inputs["v"], np.float32)
    mask = np.asarray(inputs["attn_mask"])
    Wq, bq = np.asarray(inputs["Wq"], np.float32), np.asarray(inputs["bq"], np.float32)
    Wk, bk = np.asarray(inputs["Wk"], np.float32), np.asarray(inputs["bk"], np.float32)
    Wv, bv = np.asarray(inputs["Wv"], np.float32), np.asarray(inputs["bv"], np.float32)
    Wo, bo = np.asarray(inputs["Wo"], np.float32), np.asarray(inputs["bo"], np.float32)
    gamma, beta = np.asarray(inputs["gamma"], np.float32), np.asarray(inputs["beta"], np.float32)

    qs = slice(qh * TQ, (qh + 1) * TQ)
    qb = q[b, qs, :]                       # [TQ, D]

    def ext_T(x_t):  # [D, N] -> [D+1, N] with ones row
        return np.concatenate([x_t, np.ones((1, x_t.shape[1]), np.float32)], axis=0)

    def ext_W(W, bias):  # [D, N] -> [D+1, N] with bias row
        return np.concatenate([W, bias[None, :]], axis=0)

    # Wv extended with per-head ones column: col h*65+64 gets bias 1, weights 0
    Wv_ext = np.zeros((D, VEXT), np.float32)
    bv_ext = np.zeros((VEXT,), np.float32)
    for h in range(H):
        Wv_ext[:, h * 65:h * 65 + 64] = Wv[:, h * 64:(h + 1) * 64]
        bv_ext[h * 65:h * 65 + 64] = bv[h * 64:(h + 1) * 64]
        bv_ext[h * 65 + 64] = 1.0

    return {
        "qT": ext_T(qb.T.copy()).astype(bf),
        "kT": ext_T(k[b].T.copy()).astype(bf),
        "vT": ext_T(v[b].T.copy()).astype(bf),
        "wq": ext_W(Wq, bq).astype(bf),
        "wk": ext_W(Wk, bk).astype(bf),
        "wv": ext_W(Wv_ext, bv_ext).astype(bf),
        "wo": ext_W(Wo, bo).astype(bf),
        "maskT": np.ascontiguousarray(mask[b, qs, :].T).astype(bf),
        "qres": np.ascontiguousarray(qb).astype(bf),
        "ident": np.eye(128, dtype=np.float32).astype(bf),
        "gam": gamma[None, :].copy(),
        "bet": beta[None, :].copy(),
    }


def kernel(**inputs) -> np.ndarray:
    global _LAST_RESULTS
    trivial_affine = (np.all(np.asarray(inputs["gamma"]) == 1.0)
                      and np.all(np.asarray(inputs["beta"]) == 0.0))
    trivial_bias = all(
        np.all(np.asarray(inputs[k]) == 0.0) for k in ("bq", "bk", "bv", "bo"))
    nc = bacc.Bacc("TRN2", debug=False, num_devices=NCORES)
    build_program(nc, trivial_affine=trivial_affine, trivial_bias=trivial_bias)
    nc.finalize()

    ncores_run = int(os.environ.get("KERNEL_NCORES", str(NCORES)))
    in_maps = [_prep_core_inputs(inputs, c // 2, c % 2) for c in range(NCORES)]
    trace = bool(int(os.environ.get("KERNEL_TRACE", "0")))
    res = run_bass_kernel_spmd(nc, in_maps[:ncores_run],
                               core_ids=list(range(ncores_run)), trace=trace)
    _LAST_RESULTS = {"exec_time_ns": res.exec_time_ns,
                     "profile_json": res.profile_json,
                     "res": res}

    out = np.empty((B, T, D), np.float32)
    for c in range(NCORES):
        b, qh = c // 2, c % 2
        out[b, qh * TQ:(qh + 1) * TQ, :] = res.results[c % ncores_run]["out"]
    return out


# revision 79
# speedup vs baseline: 1.0436x; 1.0001x over previous
"""Trainium2 Bass kernel for nn_MultiHeadAttention_76587856823057.

Sharding: (batch, query-half) -> 8 cores, zero collectives.
Per core: b fixed, queries TQ=1024 (half of T), all H=16 heads, all TK=2048 keys.

v4 design notes (delta over v2, which measured 620us; v4 measures ~533-542us):
 - warm-up matmuls are K=128 (full contraction rows): v2's K=1 warm-ups never
   registered as PE activity for the HAM clock gate, so the first ~32us ran
   at 1.2 GHz.  They read uninitialized qhT (no input deps, WAR only).
 - Q-proj runs as two half-contractions (ki 0-3, then 4-7 + DVE add folding
   the bf16 half-A partials back in) so it starts on the FIRST halves of the
   wq/qT DMAs (~14us); attn_sb doubles as the half-A scratch (unused until
   pass-1 kt2).  Startup is HBM-bandwidth-bound (~13MB critical set), so the
   rings are prioritized: wq/qT lead the two HWDGE rings, wk + pair-0 kT
   staging + wv + mask ride the gpsimd SWDGE ring.
 - kT staging is split load/mm with loads fired ~8 kt-iterations ahead on
   the sync ring (v2 issued them just-in-time on the PE-consuming path and
   head-of-line blocked all engines ~2x1.7us per pass).
 - kproj matmul groups run at the pass TAIL + kt3 of the next pass: they
   fill the PE idle while the last ~4 exps of the pass drain, which also
   stops the HAM clock gate re-throttling at every pass boundary.
 - pv0/pv1 merged into one [65, 2, 512] psum tile -> evac: one [1,1024]
   rowsum copy + approx-reciprocal, raw (unnormalized, scale-safe) bf16 PV
   move-out at kt2 frees the psum, one full-width gpsimd broadcast and
   in-place normalize muls at kt6.
 - khp psum evacuation on the scalar engine (DVE is the steady-state
   bottleneck: mask muls + evac ~18.8us/pass vs ACT exp 16.8us/pass).
 - Wo reuses the wk SBUF slot (allocated after pair-7 kproj, pass 13);
   phase C allocates its O-proj psum from the qk pool and is emitted right
   after pass 15 so the qh=0 output projection overlaps the tail.
 - everything else as v2: softmax shift-invariance (no global max), one mask
   multiply after exp, PE row-tiled QK (2 heads concurrent), rowsums via a
   ones-column in the V projection (M=65 PV), bf16 compute, biases via
   ones-row K=1 matmuls, software-pipelined evacuation.
 - measured engine budgets/pass (steady): DVE ~18.8us (91%), PE ~18us (87%),
   ACT exp 16x1.05us (81%); pass cadence ~20.7us.  Known remaining headroom:
   K/V-proj duplication across the q-half core pairs (~60us PE, needs a
   pair collective), exp [128,2048] batching (needs 2 more PSUM banks).

Self-contained: hardcodes all shapes; no sibling imports.
"""

import os
import numpy as np

import concourse.bass as bass
from concourse import bacc
import concourse.mybir as mybir
from concourse.tile import TileContext
from concourse.bass_utils import run_bass_kernel_spmd

F32 = mybir.dt.float32
BF16 = mybir.dt.bfloat16
AF = mybir.ActivationFunctionType

B, T, D, H, DK = 4, 2048, 1024, 16, 64
TQ = T // 2          # queries per core
TK = T               # keys per core
NCORES = 8
NPAIR = H // 2       # 8 head pairs
NFT = D // 128       # 8 feature tiles
NKT = TK // 128      # 16 key tiles
VEXT = H * (DK + 1)  # 1040: per-head [64 v-cols + ones col]

_LAST_RESULTS = {}


def build_program(nc: bass.Bass, trivial_affine: bool = False,
                  trivial_bias: bool = False):
    # ---- per-core DRAM I/O ----
    qT = nc.dram_tensor("qT", [D + 1, TQ], BF16, kind="ExternalInput").ap()
    kT = nc.dram_tensor("kT", [D + 1, TK], BF16, kind="ExternalInput").ap()
    vT = nc.dram_tensor("vT", [D + 1, TK], BF16, kind="ExternalInput").ap()
    wq = nc.dram_tensor("wq", [D + 1, D], BF16, kind="ExternalInput").ap()
    wk = nc.dram_tensor("wk", [D + 1, D], BF16, kind="ExternalInput").ap()
    wv = nc.dram_tensor("wv", [D + 1, VEXT], BF16, kind="ExternalInput").ap()
    wo = nc.dram_tensor("wo", [D + 1, D], BF16, kind="ExternalInput").ap()
    maskT = nc.dram_tensor("maskT", [TK, TQ], BF16, kind="ExternalInput").ap()
    qres = nc.dram_tensor("qres", [TQ, D], BF16, kind="ExternalInput").ap()
    ident = nc.dram_tensor("ident", [128, 128], BF16, kind="ExternalInput").ap()
    gam = nc.dram_tensor("gam", [1, D], F32, kind="ExternalInput").ap()
    bet = nc.dram_tensor("bet", [1, D], F32, kind="ExternalInput").ap()
    out = nc.dram_tensor("out", [TQ, D], F32, kind="ExternalOutput").ap()

    with TileContext(nc) as tc:
        import contextlib
        with contextlib.ExitStack() as ctx:
            pers = ctx.enter_context(tc.tile_pool(name="pers", bufs=1))

            qhT = pers.tile([128, NFT, TQ], BF16)        # 16 KB/part
            vh_sb = pers.tile([128, NKT, VEXT], BF16)    # 32.5 KB/part
            mk = pers.tile([128, NKT, TQ], BF16)         # 32 KB/part
            attn_sb = pers.tile([128, NPAIR, TQ], BF16)  # 16 KB/part
            ones = pers.tile([1, 512], BF16)             # ones (bias mms)
            ident_sb = pers.tile([128, 128], BF16)       # residual-add mm
            nc.sync.dma_start(out=ident_sb, in_=ident)

            # PSUM pools (8 banks): qk 3x2 + pvpair 1x2
            qkps = ctx.enter_context(
                tc.tile_pool(name="qkps", bufs=3, space="PSUM"))
            pvps = ctx.enter_context(
                tc.tile_pool(name="pvps", bufs=1, space="PSUM"))

            # kw outlives the attention pools: its slot is recycled for Wo
            kw = ctx.enter_context(tc.tile_pool(name="kwpool", bufs=1))

            asbuf = ctx.enter_context(contextlib.ExitStack())
            pepool = asbuf.enter_context(tc.tile_pool(name="pepool", bufs=4))
            pmpool = asbuf.enter_context(tc.tile_pool(name="pmpool", bufs=6))
            evpool = asbuf.enter_context(tc.tile_pool(name="evpool", bufs=1))
            kqpool = asbuf.enter_context(tc.tile_pool(name="kqpool", bufs=8))
            khpool = asbuf.enter_context(tc.tile_pool(name="khpool", bufs=2))
            vstack = contextlib.ExitStack()
            vw = vstack.enter_context(
                tc.tile_pool(name="vwpool", bufs=1, side="right"))
            vstage = vstack.enter_context(
                tc.tile_pool(name="vstage", bufs=16, side="right"))

            nc.vector.memset(ones, 1.0)

            # HAM warm-up: K=128 garbage matmuls (uninitialized qhT reads, no
            # input deps) register as sustained PE activity (~3.4us flips the
            # clock gate to 8/8) and bridge the first staging DMAs.
            wtile = qkps.tile([128, 1024], F32, tag="qk", name="warm")
            for _ in range(20):
                nc.tensor.matmul(wtile[0:64, 0:512], qhT[:, 1, 0:64],
                                 qhT[:, 0, 0:512], start=True, stop=True)

            # ---------------- K / V projection emitters ---------------------
            # kT staging is split load/mm: the 8 [128,512] staging DMAs for a
            # quarter are issued well before their matmuls so the PE FIFO
            # never head-of-line blocks on them (v2's dead zones).  Steady-
            # state loads ride the sync ring (keeping the ACT engine queue
            # free of DMA triggers); startup loads ride the gpsimd ring.
            def emit_kproj_load(qtr, eng=None):
                eng = eng if eng is not None else nc.sync
                qs = slice(qtr * 512, (qtr + 1) * 512)
                tiles = []
                for ki in range(NFT):
                    kq_t = kqpool.tile([128, 512], BF16, tag="ktq", name="ktq")
                    eng.dma_start(
                        out=kq_t, in_=kT[ki * 128:(ki + 1) * 128, qs])
                    tiles.append(kq_t)
                return tiles

            def emit_kproj_mm(j, khp, qtr, tiles):
                # khp[:, qtr] = (k @ Wk + bk).T rows j*128.., key qtr slice
                qs = slice(qtr * 512, (qtr + 1) * 512)
                ps_t = qkps.tile([128, 1024], F32, tag="qk", name="qk")
                ps = ps_t[:, 0:512]
                fs = slice(j * 128, (j + 1) * 128)
                for ki in range(NFT):
                    nc.tensor.matmul(ps, wk_m[:, ki, fs], tiles[ki],
                                     start=(ki == 0),
                                     stop=(trivial_bias and ki == NFT - 1))
                if not trivial_bias:
                    nc.tensor.matmul(ps, wk_b[0:1, fs], ones[0:1, 0:512],
                                     start=False, stop=True)
                # khp evac on the ACT engine: DVE is the steady-state
                # bottleneck (mask muls + evac), ACT has ~2us/pass slack
                nc.scalar.copy(khp[:, qs], ps)

            VCH = [(0, 512), (512, 1024), (1024, VEXT)]

            def emit_vproj_load(ti):
                tsl = slice(ti * 128, (ti + 1) * 128)
                vts = []
                for ki in range(NFT):
                    vt = vstage.tile([128, 128], BF16, tag="vT_m", name="vTm")
                    nc.scalar.dma_start(
                        out=vt, in_=vT[ki * 128:(ki + 1) * 128, tsl])
                    vts.append(vt)
                return vts

            def emit_vproj_mm(ti, vts):
                # vh_sb[:, ti, :] = (v @ Wv_ext + bv_ext).T tile ti (128 keys)
                tsl = slice(ti * 128, (ti + 1) * 128)
                for (c0, c1) in VCH:
                    ps_t = qkps.tile([128, 1024], F32, tag="qk", name="qk")
                    ps = ps_t[:, 0:512]
                    n = c1 - c0
                    for ki in range(NFT):
                        nc.tensor.matmul(ps[:, 0:n], vts[ki],
                                         wv_m[:, ki, c0:c1],
                                         start=(ki == 0), stop=False)
                    nc.tensor.matmul(ps[:, 0:n], ones[0:1, 0:128],
                                     wv_b[0:1, c0:c1], start=False, stop=True)
                    if c0 == 0:
                        nc.scalar.copy(vh_sb[:, ti, c0:c1], ps[:, 0:n])
                    else:
                        nc.vector.tensor_copy(vh_sb[:, ti, c0:c1], ps[:, 0:n])

            # --------- startup: kproj pair 0 + Q-proj, interleaved ----------
            # kproj qtr0 needs only wk-h0 + one staged group (2 MB): it runs
            # first, right behind the warm-up; Q-proj chunks follow as their
            # DMA halves land; remaining kproj qtrs fill between them.
            khps = [None] * NPAIR
            khps[0] = khpool.tile([128, TK], BF16, tag="khp", name="khp")
            kq_tiles = {}

            # DMA priority: wq0/qT0 lead the two HWDGE rings so the Q-proj
            # half-A matmuls start ~14us in; kproj staging + wk ride the
            # gpsimd SWDGE ring (wk0, kq0-3), then wv and the mask.
            wk_m = kw.tile([128, NFT, D], BF16, tag="wk_m")

            with tc.tile_pool(name="qppool", bufs=1, side="right") as qp:
                wq_m = qp.tile([128, NFT, D], BF16, tag="wq_m")
                qT_m = qp.tile([128, NFT, TQ], BF16, tag="qT_m")
                nc.sync.dma_start(
                    out=wq_m[:, 0:4, :],
                    in_=wq[0:512, :].rearrange("(k p) f -> p k f", p=128))
                nc.scalar.dma_start(
                    out=qT_m[:, 0:4, :],
                    in_=qT[0:512, :].rearrange("(k p) t -> p k t", p=128))
                nc.sync.dma_start(
                    out=wq_m[:, 4:8, :],
                    in_=wq[512:D, :].rearrange("(k p) f -> p k f", p=128))
                nc.scalar.dma_start(
                    out=qT_m[:, 4:8, :],
                    in_=qT[512:D, :].rearrange("(k p) t -> p k t", p=128))
                nc.gpsimd.dma_start(
                    out=wk_m[:, :, 0:512],
                    in_=wk[0:D, 0:512].rearrange("(k p) f -> p k f", p=128))
                kq_tiles[0] = emit_kproj_load(0, nc.gpsimd)
                nc.sync.dma_start(
                    out=wk_m[:, :, 512:D],
                    in_=wk[0:D, 512:D].rearrange("(k p) f -> p k f", p=128))
                if not trivial_bias:
                    wq_b = qp.tile([1, D], BF16, tag="wq_b")
                    nc.sync.dma_start(out=wq_b, in_=wq[D:D + 1, :])
                    wk_b = kw.tile([1, D], BF16, tag="wk_b")
                    nc.sync.dma_start(out=wk_b, in_=wk[D:D + 1, :])

                # Q-proj as two half-contractions: half A (ki 0-3) runs on
                # the first wq/qT DMA halves, half B accumulates in psum and
                # a DVE add folds the bf16 half-A partials back in.  attn_sb
                # (first written at pass-1 kt2) doubles as half-A scratch.
                qh_a = attn_sb

                def emit_qproj_half(half, c):
                    cs = slice(c * 512, (c + 1) * 512)
                    k0 = 4 * half
                    for fi in range(NFT):
                        fs = slice(fi * 128, (fi + 1) * 128)
                        ps_t = qkps.tile([128, 1024], F32, tag="qk", name="qk")
                        ps = ps_t[:, 0:512]
                        for ki in range(k0, k0 + 4):
                            stop = (ki == k0 + 3) and (half == 0 or trivial_bias)
                            nc.tensor.matmul(ps, wq_m[:, ki, fs], qT_m[:, ki, cs],
                                             start=(ki == k0), stop=stop)
                        if half == 0:
                            if fi % 2 == 0:
                                nc.scalar.copy(qh_a[:, fi, cs], ps)
                            else:
                                nc.vector.tensor_copy(qh_a[:, fi, cs], ps)
                        else:
                            if not trivial_bias:
                                nc.tensor.matmul(ps, wq_b[0:1, fs],
                                                 ones[0:1, 0:512],
                                                 start=False, stop=True)
                            nc.vector.tensor_add(qhT[:, fi, cs], ps,
                                                 qh_a[:, fi, cs])

                emit_qproj_half(0, 0)
                emit_qproj_half(0, 1)
                emit_kproj_mm(0, khps[0], 0, kq_tiles.pop(0))
                kq_tiles[1] = emit_kproj_load(1, nc.gpsimd)
                emit_qproj_half(1, 0)
                emit_kproj_mm(0, khps[0], 1, kq_tiles.pop(1))
                kq_tiles[2] = emit_kproj_load(2, nc.gpsimd)
                emit_qproj_half(1, 1)

            # wv + mask stream behind the staging groups on the gpsimd ring
            wv_m = vw.tile([128, NFT, VEXT], BF16, tag="wv_m")
            wv_b = vw.tile([1, VEXT], BF16, tag="wv_b")
            nc.gpsimd.dma_start(
                out=wv_m, in_=wv[0:D, :].rearrange("(k p) f -> p k f", p=128))
            nc.gpsimd.dma_start(out=wv_b, in_=wv[D:D + 1, :])
            for mc in range(4):
                ts = slice(mc * 512, (mc + 1) * 512)
                nc.gpsimd.dma_start(
                    out=mk[:, 4 * mc:4 * mc + 4, :],
                    in_=maskT[ts, :].rearrange("(t p) q -> p t q", p=128))

            emit_kproj_mm(0, khps[0], 2, kq_tiles.pop(2))
            kq_tiles[3] = emit_kproj_load(3, nc.gpsimd)
            emit_kproj_mm(0, khps[0], 3, kq_tiles.pop(3))

            vloads = {0: emit_vproj_load(0), 1: emit_vproj_load(1)}
            emit_vproj_mm(0, vloads.pop(0))
            vloads[2] = emit_vproj_load(2)
            emit_vproj_mm(1, vloads.pop(1))

            pend_evac = [None]
            pend_norm = [None]
            # kproj group deferred from the previous pass end: (jn, qtr, tiles)
            pend_kmm = [None]

            def run_pass(pi):
                j, qh = pi // 2, pi % 2
                khp = khps[j]
                qsl = slice(qh * 512, (qh + 1) * 512)
                h0sl = slice((2 * j) * 65, (2 * j) * 65 + 65)
                h1sl = slice((2 * j + 1) * 65, (2 * j + 1) * 65 + 65)
                lag = 2 if pi == 0 else 3
                pvpair = pvps.tile([65, 2, 512], F32, tag="pv", name="pv")
                pv0 = pvpair[:, 0, :]
                pv1 = pvpair[:, 1, :]
                work = []

                def emit_pv():
                    pm0, pm1, kt = work.pop(0)
                    nc.tensor.matmul(pv0, vh_sb[:, kt, h0sl], pm0,
                                     start=(kt == 0), stop=(kt == NKT - 1))
                    nc.tensor.matmul(pv1, vh_sb[:, kt, h1sl], pm1,
                                     start=(kt == 0), stop=(kt == NKT - 1))

                qks = {}

                def emit_qk(kt):
                    tsl = slice(kt * 128, (kt + 1) * 128)
                    qk = qkps.tile([128, 1024], F32, tag="qk", name="qk")
                    # concurrent row-tiled QK: h-even rows 0-63, h-odd 64-127
                    nc.tensor.matmul(qk[:, 0:512], khp[0:64, tsl],
                                     qhT[0:64, j, qsl], start=True, stop=True)
                    nc.tensor.matmul(qk[:, 512:1024], khp[64:128, tsl],
                                     qhT[64:128, j, qsl], start=True, stop=True)
                    qks[kt] = qk

                emit_qk(0)
                for kt in range(NKT):
                    # QK one iteration ahead: keeps the exp stream in front
                    # of interleaved kproj/vproj/PV work in the PE FIFO
                    if kt + 1 < NKT:
                        emit_qk(kt + 1)
                    qk = qks.pop(kt)
                    pe = pepool.tile([128, 1024], BF16, tag="pe", name="pe")
                    nc.scalar.activation(pe, qk, AF.Exp)
                    pm0 = pmpool.tile([128, 512], BF16, tag="pm", name="pm")
                    pm1 = pmpool.tile([128, 512], BF16, tag="pm", name="pm")
                    nc.vector.tensor_mul(pm0, pe[:, 0:512], mk[:, kt, qsl])
                    nc.vector.tensor_mul(pm1, pe[:, 512:1024], mk[:, kt, qsl])
                    work.append((pm0, pm1, kt))
                    # software-pipelined evac of the previous pass, two-phase:
                    # rowsum recip at kt2, normalized move-out at kt4
                    if kt == 2 and pend_evac[0] is not None:
                        pend_evac[0]()
                        pend_evac[0] = None
                    if kt == 6 and pend_norm[0] is not None:
                        pend_norm[0]()
                        pend_norm[0] = None
                    # interleaved projection work (fills PE slack)
                    if pi == 0 and kt < 14:
                        if kt < 13:
                            vloads[kt + 3] = emit_vproj_load(kt + 3)
                        emit_vproj_mm(kt + 2, vloads.pop(kt + 2))
                    # deferred kproj group from the previous pass boundary
                    # (staged load fired at that boundary, ~2.5us ago)
                    if kt == 3 and pend_kmm[0] is not None:
                        jn, qtr, tiles = pend_kmm[0]
                        pend_kmm[0] = None
                        emit_kproj_mm(jn, khps[jn], qtr, tiles)
                    # stage the next pair's group consumed at this pass's end
                    if kt == 7 and j < NPAIR - 1:
                        qa = 0 if pi % 2 == 0 else 2
                        kq_tiles[qa] = emit_kproj_load(qa)
                    tgt = min(lag, NKT - 1 - kt)
                    while len(work) > tgt:
                        emit_pv()
                while work:
                    emit_pv()
                # pass tail: kproj group A fills the PE while the exp tail
                # drains (keeps HAM warm across the boundary); group B's
                # staging fires now and its matmuls run at kt3 of the next
                # pass, bridging the boundary.
                if j < NPAIR - 1:
                    qa = 0 if pi % 2 == 0 else 2
                    qb = qa + 1
                    if khps[j + 1] is None:
                        khps[j + 1] = khpool.tile([128, TK], BF16,
                                                  tag="khp", name="khp")
                    emit_kproj_mm(j + 1, khps[j + 1], qa, kq_tiles.pop(qa))
                    kq_tiles[qb] = emit_kproj_load(qb)
                    pend_kmm[0] = (j + 1, qb, kq_tiles.pop(qb))

                def evac():
                    # pvpair-critical phase: rowsums + raw (unnormalized) PV
                    # out of the psum; bf16 attn is scale-invariant-safe
                    rs = evpool.tile([1, 2, 512], F32, tag="rs", name="rs")
                    nc.vector.tensor_copy(rs, pvpair[64:65, :, :])
                    nc.vector.tensor_copy(attn_sb[0:64, j, qsl],
                                          pvpair[0:64, 0, :])
                    nc.vector.tensor_copy(attn_sb[64:128, j, qsl],
                                          pvpair[0:64, 1, :])

                    def norm():
                        nc.vector.reciprocal_approx_fast(rs, rs)
                        rrb = evpool.tile([128, 2, 512], F32, tag="rrb",
                                          name="rrb")
                        nc.gpsimd.partition_broadcast(rrb, rs)
                        for hh in (0, 1):
                            sl = attn_sb[64 * hh:64 * hh + 64, j, qsl]
                            nc.vector.tensor_mul(
                                sl, sl, rrb[64 * hh:64 * hh + 64, hh, :])
                    pend_norm[0] = norm
                pend_evac[0] = evac

            for pi in range(2 * NPAIR):
                run_pass(pi)
                if pi == 0:
                    vstack.close()
                if pi == 13:
                    # Wk's last reader (pair-7 kproj) has been emitted; its
                    # kw slot is recycled for Wo (WAR dep, DMA runs ~pass 14)
                    wo_m = kw.tile([128, NFT, D], BF16, tag="wk_m")
                    nc.sync.dma_start(
                        out=wo_m[:, :, 0:512],
                        in_=wo[0:D, 0:512].rearrange("(k p) f -> p k f", p=128))
                    nc.sync.dma_start(
                        out=wo_m[:, :, 512:1024],
                        in_=wo[0:D, 512:1024].rearrange("(k p) f -> p k f", p=128))
                    if not trivial_bias:
                        wo_b = kw.tile([1, D], BF16, tag="wk_b")
                        nc.sync.dma_start(out=wo_b, in_=wo[D:D + 1, :])
            pend_evac[0]()
            pend_evac[0] = None
            pend_norm[0]()
            pend_norm[0] = None
            # free attention-phase SBUF pools (space reused by phase C pools;
            # WAR deps keep the pass-15 tail and evac correct)
            asbuf.close()

            # ------------ phase C: out-proj + residual + LN -----------------
            # O-proj psum comes from the qk pool; emitted right after pass 15
            # so qh=0 projection overlaps the last evac / pass tail.
            with tc.tile_pool(name="cq", bufs=2) as cq, \
                 tc.tile_pool(name="cl", bufs=2) as cl:

                eps_t = cl.tile([128, 1], F32, tag="eps")
                nc.vector.memset(eps_t, 1e-5)
                if not trivial_affine:
                    gam_r = cl.tile([1, D], F32, tag="gam_r")
                    bet_r = cl.tile([1, D], F32, tag="bet_r")
                    nc.sync.dma_start(out=gam_r, in_=gam)
                    nc.sync.dma_start(out=bet_r, in_=bet)
                    gam_b = cl.tile([128, D], F32, tag="gam_b")
                    bet_b = cl.tile([128, D], F32, tag="bet_b")
                    nc.gpsimd.partition_broadcast(gam_b, gam_r)
                    nc.gpsimd.partition_broadcast(bet_b, bet_r)

                for qt in range(NFT):
                    qts = slice(qt * 128, (qt + 1) * 128)
                    qres_t = cq.tile([128, D], BF16, tag="qres")
                    nc.scalar.dma_start(out=qres_t, in_=qres[qts, :])
                    ps = qkps.tile([128, 1024], F32, tag="qk", name="oproj")
                    for c in range(2):
                        cs = slice(c * 512, (c + 1) * 512)
                        for ki in range(NFT):
                            nc.tensor.matmul(ps[:, cs], attn_sb[:, ki, qts],
                                             wo_m[:, ki, cs],
                                             start=(ki == 0), stop=False)
                        if not trivial_bias:
                            nc.tensor.matmul(ps[:, cs], ones[0:1, 0:128],
                                             wo_b[0:1, cs], start=False,
                                             stop=False)
                        # residual folded into the psum via identity matmul
                        # (frees the DVE add in the DVE-bound tail)
                        nc.tensor.matmul(ps[:, cs], ident_sb, qres_t[:, cs],
                                         start=False, stop=True)

                    stats = cl.tile([128, 2, 6], F32, tag="stats")
                    nc.vector.bn_stats(stats[:, 0, :], ps[:, 0:512])
                    nc.vector.bn_stats(stats[:, 1, :], ps[:, 512:1024])
                    mv = cl.tile([128, 2], F32, tag="mv")
                    nc.vector.bn_aggr(mv, stats)
                    sq = cl.tile([128, 1], F32, tag="sq")
                    nc.scalar.activation(sq, mv[:, 1:2], AF.Sqrt, bias=eps_t)
                    rstd = cl.tile([128, 1], F32, tag="rstd")
                    nc.vector.reciprocal(rstd, sq)
                    xo = cl.tile([128, D], F32, tag="xo")
                    nc.vector.tensor_scalar(xo, ps, mv[:, 0:1], rstd,
                                            op0=mybir.AluOpType.subtract,
                                            op1=mybir.AluOpType.mult)
                    if not trivial_affine:
                        nc.vector.tensor_mul(xo, xo, gam_b)
                        nc.vector.tensor_add(xo, xo, bet_b)
                    nc.sync.dma_start(out=out[qts, :], in_=xo)
    return nc


def _prep_core_inputs(inputs, b, qh):
    """Build the per-core input map (host-side layout prep only)."""
    import ml_dtypes
    bf = ml_dtypes.bfloat16
    q = np.asarray(inputs["q"], np.float32)
    k = np.asarray(inputs["k"], np.float32)
    v = np.asarray(inputs["v"], np.float32)
    mask = np.asarray(inputs["attn_mask"])
    Wq, bq = np.asarray(inputs["Wq"], np.float32), np.asarray(inputs["bq"], np.float32)
    Wk, bk = np.asarray(inputs["Wk"], np.float32), np.asarray(inputs["bk"], np.float32)
    Wv, bv = np.asarray(inputs["Wv"], np.float32), np.asarray(inputs["bv"], np.float32)
    Wo, bo = np.asarray(inputs["Wo"], np.float32), np.asarray(inputs["bo"], np.float32)
    gamma, beta = np.asarray(inputs["gamma"], np.float32), np.asarray(inputs["beta"], np.float32)

    qs = slice(qh * TQ, (qh + 1) * TQ)
    qb = q[b, qs, :]                       # [TQ, D]

    def ext_T(x_t):  # [D, N] -> [D+1, N] with ones row
        return np.concatenate([x_t, np.ones((1, x_t.shape[1]), np.float32)], axis=0)

    def ext_W(W, bias):  # [D, N] -> [D+1, N] with bias row
        return np.concatenate([W, bias[None, :]], axis=0)

    # Wv extended with per-head ones column: col h*65+64 gets bias 1, weights 0
    Wv_ext = np.zeros((D, VEXT), np.float32)
    bv_ext = np.zeros((VEXT,), np.float32)
    for h in range(H):
        Wv_ext[:, h * 65:h * 65 + 64] = Wv[:, h * 64:(h + 1) * 64]
        bv_ext[h * 65:h * 65 + 64] = bv[h * 64:(h + 1) * 64]
        bv_ext[h * 65 + 64] = 1.0

    return {
        "qT": ext_T(qb.T.copy()).astype(bf),
        "kT": ext_T(k[b].T.copy()).astype(bf),
        "vT": ext_T(v[b].T.copy()).astype(bf),
        "wq": ext_W(Wq, bq).astype(bf),
        "wk": ext_W(Wk, bk).astype(bf),
        "wv": ext_W(Wv_ext, bv_ext).astype(bf),
        "wo": ext_W(Wo, bo).astype(bf),
        "maskT": np.ascontiguousarray(mask[b, qs, :].T).astype(bf),
        "qres": np.ascontiguousarray(qb).astype(bf),
        "ident": np.eye(128, dtype=np.float32).astype(bf),
        "gam": gamma[None, :].copy(),
        "bet": beta[None, :].copy(),
    }


def kernel(**inputs) -> np.ndarray:
    global _LAST_RESULTS
    trivial_affine = (np.all(np.asarray(inputs["gamma"]) == 1.0)
                      and np.all(np.asarray(inputs["beta"]) == 0.0))
    trivial_bias = all(
        np.all(np.asarray(inputs[k]) == 0.0) for k in ("bq", "bk", "bv", "bo"))
    nc = bacc.Bacc("TRN2", debug=False, num_devices=NCORES)
    build_program(nc, trivial_affine=trivial_affine, trivial_bias=trivial_bias)
    nc.finalize()

    ncores_run = int(os.environ.get("KERNEL_NCORES", str(NCORES)))
    in_maps = [_prep_core_inputs(inputs, c // 2, c % 2) for c in range(NCORES)]
    trace = bool(int(os.environ.get("KERNEL_TRACE", "0")))
    res = run_bass_kernel_spmd(nc, in_maps[:ncores_run],
                               core_ids=list(range(ncores_run)), trace=trace)
    _LAST_RESULTS = {"exec_time_ns": res.exec_time_ns,
                     "profile_json": res.profile_json,
                     "res": res}

    out = np.empty((B, T, D), np.float32)
    for c in range(NCORES):
        b, qh = c // 2, c % 2
        out[b, qh * TQ:(qh + 1) * TQ, :] = res.results[c % ncores_run]["out"]
    return out
